# revision 33
# baseline (speedup 1.0000x reference)
"""Multi-head causal self-attention with RoPE on 8 Trainium2 NeuronCores.

Sharding: (batch, head-group) data+tensor parallel. Core c handles batch
c//4 and heads [3*(c%4), 3*(c%4)+3). Each core runs fused
QKV-projection + RoPE + causal attention + output-projection and emits a
partial transposed [D, S] output (fp16); the host sums the 4 head-group
partials per batch in fp32.

Device-side structure:
  - Q/K transposes to [d, s] go through the DMA X-bar: ONE [128,512]
    transpose per seq chunk fanning into a single [128, 4*S] tile of
    head-pair blocks (on the sync queue — a queued DMA holds its
    sequencer while waiting on deps, so it must not sit in front of
    the scalar engine's exps).
  - W_qkv columns are ordered (q0 q1 | k0 k1 | q2 k2 | v0 v1 v2) so the
    transposed blocks land as stacked head pairs; score matmuls for a
    pair run on PE row-groups 0-1/2-3 (K=64 row tiling) back-to-back;
    head 2 gets a swapped duplicate [k2;q2] so its blocks alternate row
    groups by k-block parity.
  - Score matmuls are emitted as <=256-column pieces: HW-measured
    per-matmul cost collapses from ~300ns (full 512-col fp32 bank) to
    ~40-100ns for sub-bank widths with alternating row groups.
  - RoPE runs in bf16 from an SBUF copy (DVE 16-bit rate) with bf16
    cos/sin tables.
  - Scores are computed transposed (S^T[k, q]); softmax skips
    max-subtraction; denominator comes free from a ones-column in V
    (M=65 PV matmuls — padding to M=128 measured slower, keep 65).
  - Causality: diagonal blocks compute only the needed (shrunk) column
    range; the partial triangle is zeroed AFTER exp in SBUF bf16 by
    GPSIMD affine_select restricted to the 128 columns where the mask
    has any effect (beyond 128 cols past the block diagonal, qi >= k
    always holds; fully-masked 128-col spans of a2's j=1 blocks are
    never read by PV, which starts at its own offset) — no mask work
    on the PSUM path and ~4x less than masking the whole block.
  - Output projection runs transposed: outT[d_model, s] accumulates
    Wo-chunk-stationary matmuls over ctxT slabs (ctx is already stored
    transposed), 2 matmuls per (qt, dmodel-group) unit; partials leave
    as [D, S] fp16 and the host transposes while summing.
  - Phase-1 chunks and phase-3 output-projection units are woven into
    the attention emission as PE filler, with each attention unit's
    tail (PV flush + normalize) deferred past the next unit's first
    k-blocks so neither PE nor ACT drains at unit boundaries.
  - P1 chunks drain just-in-time (p1late): chunk c is emitted ~one
    unit before its first consuming k-block instead of two units
    early. This spreads PE filler into the big late query tiles where
    the ACT engine is otherwise the rail, and was worth ~35us on HW
    (the early-flush schedule bunches p1 into the small early units
    and starves the qt=2/3 stretches).
  - The two reciprocal->broadcast->multiply normalize chains of an a1
    tail are emitted interleaved so their cross-engine hops pipeline.

HW-measured cost notes (this container, loop-delta timing):
  score matmuls in alternating 64-row groups genuinely overlap (two
  256-col pieces ~ cost of one); exp on ACT is ~1.33 ns/col (the
  ~82us exp rail and ~100us PE rail bound the kernel at ~105us ideal
  overlap); a cross-engine dependency round-trip is ~300-550ns, which
  is why emission order, not engine busy, dominates the schedule.
"""

import numpy as np

import concourse.bass as bass
import concourse.tile as tile
from concourse import bacc, mybir
from concourse._compat import with_exitstack
from concourse.bass_utils import run_bass_kernel_spmd

# Problem constants (hardcoded; kernel.py must be self-contained).
B = 2
S = 2048
D_MODEL = 768
NUM_HEADS = 12
HD = 64  # head dim
ROPE_THETA = 10000.0
MAX_SEQ_LEN = 2048

N_CORES = 8
HG = 3  # heads per core
E = 3 * HG * HD  # 576 qkv rows per core
P = 128
NSC = S // P  # 16 seq chunks
NKC = D_MODEL // P  # 6 d_model chunks
NG = D_MODEL // P  # 6 output d_model groups
F = HD // 2  # 32 rope freqs
QB = 512  # query block
NQT = S // QB  # 4 query tiles
VW = HD + 1  # V block width incl. ones column

F32 = mybir.dt.float32
F16 = mybir.dt.float16
MM = mybir.dt.bfloat16
EXP = mybir.ActivationFunctionType.Exp
GE = mybir.AluOpType.is_ge


@with_exitstack
def emit_mhsa(ctx, tc, loop_m=1, py_unroll=False, qkb_eng="act", ob_eng="dve",
              swap_eng="dve", parts="p1,att,p3", keep1=5, keep2=2, pace=2,
              pspool="shared", expsplit=0, odma="sync", obufs=6, rbufs=4,
              p1late=1, wide=0):
    parts = set(parts.split(","))
    nc = tc.nc
    xT = nc.dram_tensor("xT", [D_MODEL, S], MM, kind="ExternalInput").ap()
    wqkvT = nc.dram_tensor("wqkvT", [D_MODEL, E], MM, kind="ExternalInput").ap()
    woT = nc.dram_tensor("woT", [HG * HD, D_MODEL], MM, kind="ExternalInput").ap()
    cosg = nc.dram_tensor("cosg", [S, F], MM, kind="ExternalInput").ap()
    sing = nc.dram_tensor("sing", [S, F], MM, kind="ExternalInput").ap()
    out = nc.dram_tensor("outT_partial", [D_MODEL, S], F16, kind="ExternalOutput").ap()

    const = ctx.enter_context(tc.tile_pool(name="const", bufs=1))
    persist = ctx.enter_context(tc.tile_pool(name="persist", bufs=1))

    # ---- constants & weights ----
    cos_sb = const.tile([P, NSC * F], MM, tag="cos")
    sin_sb = const.tile([P, NSC * F], MM, tag="sin")
    nc.sync.dma_start(
        cos_sb[:].rearrange("p (n f) -> p n f", f=F),
        cosg.rearrange("(n p) f -> p n f", p=P),
    )
    nc.sync.dma_start(
        sin_sb[:].rearrange("p (n f) -> p n f", f=F),
        sing.rearrange("(n p) f -> p n f", p=P),
    )

    w_sb = []
    for kc in range(NKC):
        w = const.tile([P, E], MM, tag=f"wqkv{kc}", name=f"wqkv{kc}")
        nc.sync.dma_start(w[:], wqkvT[kc * P : (kc + 1) * P, :])
        w_sb.append(w)
    wo0 = const.tile([P, D_MODEL], MM, tag="wo0")
    wo1 = const.tile([HD, D_MODEL], MM, tag="wo1")
    nc.sync.dma_start(wo0[:], woT[0:P, :])
    nc.sync.dma_start(wo1[:], woT[P : HG * HD, :])

    x_sb = []
    for kc in range(NKC):
        xt = const.tile([P, S], MM, tag=f"x{kc}", name=f"x{kc}")
        nc.sync.dma_start(xt[:, 0:512], xT[kc * P : (kc + 1) * P, 0:512])
        x_sb.append(xt)
    for kc in range(NKC):
        nc.sync.dma_start(x_sb[kc][:, 512:S], xT[kc * P : (kc + 1) * P, 512:S])

    # ---- persistent intermediates ----
    # roped q/k in [d, s]: 4 stacked head-pair blocks in one tile:
    # t=0: [q0;q1]  t=1: [k0;k1]  t=2: [q2;k2]  t=3: [k2;q2]
    tqk = persist.tile([P, 4 * S], MM, tag="tqk")
    tqk4 = tqk[:].rearrange("p (t s) -> p t s", t=4)
    tq01 = tqk[:, 0:S]
    tk01 = tqk[:, S : 2 * S]
    tqk2 = tqk[:, 2 * S : 3 * S]
    tkq2 = tqk[:, 3 * S : 4 * S]
    v_sb = persist.tile([P, HG * NSC * VW], MM, tag="v")
    ctxA = persist.tile([P, S], MM, tag="ctxA")  # h0 rows 0:64, h1 rows 64:128
    ctxB = persist.tile([HD, S], MM, tag="ctxB")  # h2

    # ones columns for the PV denominator (V parts written each pass)
    v4 = v_sb[:].rearrange("p (h n w) -> p h n w", h=HG, n=NSC)
    nc.gpsimd.memset(v4[:, :, :, HD:VW], 1.0)
    if parts != {"p1", "att", "p3"}:  # timing probes read uninitialized tiles
        nc.gpsimd.memset(tqk[:], 0.01)
        nc.gpsimd.memset(v_sb[:], 0.01)
        nc.gpsimd.memset(ctxA[:], 0.01)
        nc.gpsimd.memset(ctxB[:], 0.01)

    if loop_m > 1 and not py_unroll:  # timing builds only: repeat compute body
        ctx.enter_context(tc.For_i(0, loop_m, 1))

    if pspool == "shared":
        ps_main = ctx.enter_context(
            tc.tile_pool(name="ps_main", bufs=3, space="PSUM"))
        ps_aux = ps_main
    else:  # scores get a dedicated pool; p1/p3 fillers can't starve them
        ps_main = ctx.enter_context(
            tc.tile_pool(name="ps_main", bufs=2, space="PSUM"))
        ps_aux = ctx.enter_context(
            tc.tile_pool(name="ps_aux", bufs=1, space="PSUM"))
    aux_tag = "ps" if pspool == "shared" else "psx"
    ps_ctx = ctx.enter_context(tc.tile_pool(name="ps_ctx", bufs=2, space="PSUM"))
    rope_pool = ctx.enter_context(tc.tile_pool(name="rope", bufs=rbufs))
    pp_pool = ctx.enter_context(tc.tile_pool(name="pp", bufs=8))
    norm_pool = ctx.enter_context(tc.tile_pool(name="norm", bufs=2))
    ob_pool = ctx.enter_context(tc.tile_pool(name="ob", bufs=obufs))
    odma_start = (nc.gpsimd.dma_start if odma == "pool"
                  else nc.sync.dma_start)

    qkb_copy = nc.scalar.copy if qkb_eng == "act" else nc.vector.tensor_copy
    ob_copy = nc.scalar.copy if ob_eng == "act" else nc.vector.tensor_copy
    swap_copy = (nc.gpsimd.tensor_copy if swap_eng == "pool"
                 else nc.vector.tensor_copy)

    # ================= emission building blocks =================

    def p1_chunk(sc):
        """QKV projection + RoPE + V copy + DMA-transpose for seq chunk sc."""
        pq = ps_aux.tile([P, 1024], F32, tag=aux_tag, name=f"p1_{sc}")
        pqk = pq[:, 0:384]  # bank 0
        pv = pq[:, 512:704]  # bank 1
        for kc in range(NKC):
            lhs = x_sb[kc][:, sc * P : (sc + 1) * P]
            st, sp = kc == 0, kc == NKC - 1
            nc.tensor.matmul(pqk, lhs, w_sb[kc][:, 0:384], start=st, stop=sp)
            nc.tensor.matmul(pv, lhs, w_sb[kc][:, 384:576], start=st, stop=sp)

        # V: strided copy into the 3 per-head blocks (+ones untouched).
        v_dst = v_sb[:].rearrange("p (h n w) -> p h n w", h=HG, n=NSC)
        nc.vector.tensor_copy(
            v_dst[:, :, sc, 0:HD], pv.rearrange("p (h w) -> p h w", h=HG)
        )

        # RoPE in bf16 (DVE 16-bit rate): one PSUM->SBUF copy, 6 DVE ops.
        qkb = rope_pool.tile([P, 384], MM, tag="qkb", name=f"qkb{sc}")
        qkb_copy(qkb[:], pqk)
        cos6 = (cos_sb[:, sc * F : (sc + 1) * F]
                .unsqueeze(1).broadcast_to([P, 6, F]))
        sin6 = (sin_sb[:, sc * F : (sc + 1) * F]
                .unsqueeze(1).broadcast_to([P, 6, F]))
        ro = rope_pool.tile([P, 512], MM, tag="ro", name=f"ro{sc}")
        r4 = ro[:, 0:384].rearrange("p (t two f) -> p t two f", t=6, two=2)
        s4 = qkb[:].rearrange("p (t two f) -> p t two f", t=6, two=2)
        ev, od = s4[:, :, 0, :], s4[:, :, 1, :]
        shape = [P, 6 * F]
        t1 = rope_pool.tile(shape, MM, tag="t1")
        t2 = rope_pool.tile(shape, MM, tag="t2")
        t14 = t1[:].rearrange("p (t f) -> p t f", t=6)
        t24 = t2[:].rearrange("p (t f) -> p t f", t=6)
        nc.vector.tensor_mul(t14, ev, cos6)
        nc.vector.tensor_mul(t24, od, sin6)
        nc.vector.tensor_sub(r4[:, :, 0, :], t14, t24)
        t3 = rope_pool.tile(shape, MM, tag="t3")
        t4 = rope_pool.tile(shape, MM, tag="t4")
        t34 = t3[:].rearrange("p (t f) -> p t f", t=6)
        t44 = t4[:].rearrange("p (t f) -> p t f", t=6)
        nc.vector.tensor_mul(t34, ev, sin6)
        nc.vector.tensor_mul(t44, od, cos6)
        nc.vector.tensor_add(r4[:, :, 1, :], t34, t44)

        # swapped duplicate [k2|q2] in cols 384:512
        swap_copy(ro[:, 384:448], ro[:, 320:384])
        swap_copy(ro[:, 448:512], ro[:, 256:320])

        # one [128,512] X-bar transpose into the 4 stacked blocks.
        # Stays on the sync queue: a queued DMA holds its sequencer while
        # waiting on deps, and the scalar queue must stay free for exps.
        nc.sync.dma_start(
            tqk4[:, :, sc * P : (sc + 1) * P], ro[:], transpose=True
        )

    def norm_heads(pcs_dsts):
        """dst = pctx[0:HD] / ones-row, pipelined across heads: both
        reciprocals issue first so the DVE->Pool->DVE chains overlap."""
        rs = []
        for pc, _ in pcs_dsts:
            rinv = norm_pool.tile([1, QB], F32, tag="rinv")
            nc.vector.reciprocal(rinv[0:1, :], pc[HD : HD + 1, :])
            rs.append(rinv)
        bs = []
        for rinv, (pc, _) in zip(rs, pcs_dsts):
            rbc = norm_pool.tile([HD, QB], F32, tag="rbc")
            nc.gpsimd.partition_broadcast(rbc[:], rinv[0:1, :])
            bs.append(rbc)
        for rbc, (pc, dst) in zip(bs, pcs_dsts):
            nc.vector.tensor_mul(dst, pc[0:HD, :], rbc[:])

    def a1_qt(qt, filler):
        """Heads 0,1: row-group-paired scores + exp + PV, one query tile.

        Generator: yields "kb" after each k-block and "pretail" before the
        PV flush + normalize, so the driver can start the next unit's
        scores/exps before this unit's tail work is queued.
        """
        nb = 4 * qt + 4
        pc0 = ps_ctx.tile([VW, QB], F32, tag="pctx", name=f"pc0_{qt}")
        pc1 = ps_ctx.tile([VW, QB], F32, tag="pctx", name=f"pc1_{qt}")
        pend = []

        def pv_flush(keep):
            while len(pend) > keep:
                psb, kb, off = pend.pop(0)
                for j, pc in ((0, pc0), (1, pc1)):
                    vb = (j * NSC + kb) * VW
                    nc.tensor.matmul(
                        pc[:, off:QB], v_sb[:, vb : vb + VW],
                        psb[:, j * QB + off : (j + 1) * QB],
                        start=(kb == 0), stop=(kb == nb - 1),
                    )

        for kb in range(nb):
            m = kb - 4 * qt
            off = 128 * m if m > 0 else 0
            w = QB - off
            ks = slice(kb * P, (kb + 1) * P)
            pss = ps_main.tile([P, 1024], F32, tag="ps", name=f"a1_{qt}_{kb}")
            # split into <=256-col pieces, alternating row groups per MM:
            # sub-bank-width matmuls issue far faster than full 512s.
            wa = w if wide else ((w + 255) // 256 * 128 if w > 256 else w)
            for o0, o1 in ((off, off + wa), (off + wa, QB)):
                if o0 >= o1:
                    continue
                qs = slice(qt * QB + o0, qt * QB + o1)
                nc.tensor.matmul(
                    pss[:, o0:o1], tk01[0:HD, ks], tq01[0:HD, qs],
                    start=True, stop=True,
                )
                nc.tensor.matmul(
                    pss[:, QB + o0 : QB + o1], tk01[HD:P, ks], tq01[HD:P, qs],
                    start=True, stop=True,
                )
            psb = pp_pool.tile([P, 1024], MM, tag="psb", name=f"e1_{qt}_{kb}")
            if m >= 0:
                src = pss[:].rearrange("p (h q) -> p h q", h=2)[:, :, off:QB]
                dst = psb[:].rearrange("p (h q) -> p h q", h=2)[:, :, off:QB]
                if expsplit:
                    nc.scalar.activation(dst[:, 0], src[:, 0], EXP, scale=0.125)
                    nc.scalar.activation(dst[:, 1], src[:, 1], EXP, scale=0.125)
                else:
                    nc.scalar.activation(dst, src, EXP, scale=0.125)
                # zero upper triangle in-place on GPSIMD (keep iff qi >= k).
                # Only the first 128 region cols can violate qi >= k.
                nc.gpsimd.affine_select(
                    out=dst[:, :, 0:P], in_=dst[:, :, 0:P], compare_op=GE,
                    fill=0.0, base=0, channel_multiplier=-1,
                    pattern=[[0, 2], [1, P]],
                )
            elif expsplit:
                nc.scalar.activation(psb[:, 0:QB], pss[:, 0:QB], EXP, scale=0.125)
                nc.scalar.activation(psb[:, QB:1024], pss[:, QB:1024], EXP,
                                     scale=0.125)
            else:
                nc.scalar.activation(psb[:], pss[:], EXP, scale=0.125)
            pend.append((psb, kb, off))
            pv_flush(keep1)
            filler(1)
            yield "kb"
        yield "pretail"
        pv_flush(0)
        norm_heads([(pc0, ctxA[0:HD, qt * QB : (qt + 1) * QB]),
                    (pc1, ctxA[HD:P, qt * QB : (qt + 1) * QB])])

    def a2_qt(qt, filler):
        """Head 2: k-block pairs alternate row groups by parity. Generator
        with the same yield protocol as a1_qt."""
        nb = 4 * qt + 4
        pc2 = ps_ctx.tile([VW, QB], F32, tag="pctx", name=f"pc2_{qt}")
        pend = []

        def pv_flush(keep):
            while len(pend) > keep:
                psb, kb0 = pend.pop(0)
                for j in (0, 1):
                    kb = kb0 + j
                    mj = kb - 4 * qt
                    offj = 128 * mj if mj > 0 else 0
                    vb = (2 * NSC + kb) * VW
                    nc.tensor.matmul(
                        pc2[:, offj:QB], v_sb[:, vb : vb + VW],
                        psb[:, j * QB + offj : (j + 1) * QB],
                        start=(kb == 0), stop=(kb == nb - 1),
                    )

        for kb0 in range(0, nb, 2):
            m0 = kb0 - 4 * qt
            off = 128 * m0 if m0 > 0 else 0
            w = QB - off
            pss = ps_main.tile([P, 1024], F32, tag="ps", name=f"a2_{qt}_{kb0}")
            wa = w if wide else ((w + 255) // 256 * 128 if w > 256 else w)
            for o0, o1 in ((off, off + wa), (off + wa, QB)):
                if o0 >= o1:
                    continue
                for j in (0, 1):
                    kb = kb0 + j
                    ks = slice(kb * P, (kb + 1) * P)
                    qs = slice(qt * QB + o0, qt * QB + o1)
                    if kb % 2 == 0:
                        lhsT, rhs = tkq2[0:HD, ks], tqk2[0:HD, qs]
                    else:
                        lhsT, rhs = tqk2[HD:P, ks], tkq2[HD:P, qs]
                    nc.tensor.matmul(
                        pss[:, j * QB + o0 : j * QB + o1], lhsT, rhs,
                        start=True, stop=True,
                    )
            psb = pp_pool.tile([P, 1024], MM, tag="psb", name=f"e2_{qt}_{kb0}")
            if m0 >= 0:
                src = pss[:].rearrange("p (h q) -> p h q", h=2)[:, :, off:QB]
                dst = psb[:].rearrange("p (h q) -> p h q", h=2)[:, :, off:QB]
                nc.scalar.activation(dst, src, EXP, scale=0.125)
                # j=0 block: triangle in region cols [0:128) (keep qi >= k).
                nc.gpsimd.affine_select(
                    out=dst[:, 0, 0:P], in_=dst[:, 0, 0:P], compare_op=GE,
                    fill=0.0, base=0, channel_multiplier=-1,
                    pattern=[[1, P]],
                )
                # j=1 block: its PV starts 128 cols later, so only region
                # cols [128:256) need the mask (keep qi-128 >= k there).
                nc.gpsimd.affine_select(
                    out=dst[:, 1, P : 2 * P], in_=dst[:, 1, P : 2 * P],
                    compare_op=GE, fill=0.0, base=0, channel_multiplier=-1,
                    pattern=[[1, P]],
                )
            else:
                nc.scalar.activation(psb[:], pss[:], EXP, scale=0.125)
            pend.append((psb, kb0))
            pv_flush(keep2)
            filler(1)
            yield "kb"
        yield "pretail"
        pv_flush(0)
        norm_heads([(pc2, ctxB[:, qt * QB : (qt + 1) * QB])])

    def p3_unit(qt, g):
        """Transposed output projection: outT[g-block, qt-slab]."""
        po = ps_aux.tile([P, 1024], F32, tag=aux_tag, name=f"p3_{qt}_{g}")
        gs = slice(g * P, (g + 1) * P)
        qs = slice(qt * QB, (qt + 1) * QB)
        nc.tensor.matmul(po[:, 0:QB], wo0[:, gs], ctxA[:, qs],
                         start=True, stop=False)
        nc.tensor.matmul(po[:, 0:QB], wo1[:, gs], ctxB[:, qs],
                         start=False, stop=True)
        ob = ob_pool.tile([P, QB], F16, tag="ob")
        ob_copy(ob[:], po[:, 0:QB])
        odma_start(out[gs, qs], ob[:])

    # ================= schedule =================
    # Filler queue: PE-heavy work woven between attention k-blocks so the
    # PE keeps running while ACT chews through the exps. P1 chunks have
    # deadlines (chunk c before attention unit c//4); P3 units become
    # available after their unit completes.
    def body():
        fillers = []

        def filler(budget):
            for _ in range(budget):
                if not fillers:
                    return
                fillers.pop(0)()

        def flush_p1_until(chunk_limit):
            while (fillers and fillers[0].__name__ == "p1"
                   and fillers[0].c < chunk_limit):
                fillers.pop(0)()

        def mk_p1(c):
            def p1():
                p1_chunk(c)
            p1.__name__ = "p1"
            p1.c = c
            return p1

        def mk_p3(qt, g):
            def p3():
                p3_unit(qt, g)
            p3.__name__ = "p3"
            return p3

        if "p1" in parts:
            for sc in range(4):
                p1_chunk(sc)
            fillers.extend(mk_p1(c) for c in range(4, NSC))

        every = [0]

        def paced_filler(_):
            every[0] += 1
            if every[0] % pace == 0:
                filler(1)

        if "att" not in parts:
            while fillers:
                fillers.pop(0)()
            if "p3" in parts:
                for qt_ in range(NQT):
                    for g_ in range(NG):
                        p3_unit(qt_, g_)
            return

        # Drive the attention units, overlapping each unit's tail (PV flush
        # + normalize) past the next unit's first k-blocks so ACT never
        # drains at a unit boundary. P3 units for a query tile only enter
        # the filler queue once the a2 tail (writing their ctxB) is emitted.
        pending_tail = None  # (generator, p3_qt or None)

        def finish(tail):
            g, p3_qt = tail
            for _ in g:
                pass
            if p3_qt is not None and "p3" in parts:
                fillers.extend(mk_p3(p3_qt, g_) for g_ in range(NG))

        for qt in range(NQT):
            # emit next unit's P1 chunks now: their QKV->RoPE->transpose
            # chain completes while this unit's attention runs.  With
            # p1late, chunks instead drain just-in-time (3 k-blocks ahead
            # of first use, enforced inside the kb loop) so PE filler work
            # spreads into the big late units instead of bunching early.
            flush_p1_until(4 * (qt + 1) if p1late else 4 * (qt + 2))
            for unit, p3_qt in (
                (a1_qt(qt, paced_filler), None),
                (a2_qt(qt, paced_filler), qt),
            ):
                steps = 0
                for ev in unit:
                    if ev == "kb":
                        steps += 1
                        if p1late and p3_qt is None:
                            flush_p1_until(4 * (qt + 1) + steps)
                        if steps == 2 and pending_tail is not None:
                            finish(pending_tail)
                            pending_tail = None
                    else:  # "pretail"
                        if pending_tail is not None:
                            finish(pending_tail)
                        pending_tail = (unit, p3_qt)
                        break
        if pending_tail is not None:
            finish(pending_tail)
        while fillers:
            fillers.pop(0)()

    if py_unroll:
        for _ in range(loop_m):
            body()
    else:
        body()


_NC_CACHE = None


def build_nc(loop_m=1, **kw):
    global _NC_CACHE
    key = (loop_m, tuple(sorted(kw.items())))
    if _NC_CACHE is None or getattr(_NC_CACHE, "_key", None) != key:
        nc = bacc.Bacc("TRN2", target_bir_lowering=False, debug=False)
        with tile.TileContext(nc) as tc:
            emit_mhsa(tc, loop_m=loop_m, **kw)
        nc.compile()
        nc._key = key
        _NC_CACHE = nc
    return _NC_CACHE


def _rope_tables():
    powers = np.arange(0, HD, 2, dtype=np.float32) / np.float32(HD)
    freqs = (1.0 / (ROPE_THETA ** powers)).astype(np.float32)
    t = np.arange(MAX_SEQ_LEN, dtype=np.float32)
    ang = t[:, None] * freqs[None, :]
    return np.cos(ang).astype(np.float32), np.sin(ang).astype(np.float32)


def host_inputs(x, token_positions, W_qkv, W_o):
    """Build the 8 per-core input maps (shard + layout prep)."""
    import ml_dtypes

    x = np.asarray(x, dtype=np.float32)
    token_positions = np.asarray(token_positions)
    W_qkv = np.asarray(W_qkv, dtype=np.float32)
    W_o = np.asarray(W_o, dtype=np.float32)

    cos_t, sin_t = _rope_tables()
    # De-interleave head-dim rows of W_q/W_k so RoPE pairs become
    # contiguous 32-wide halves on device.
    perm = np.concatenate([np.arange(0, HD, 2), np.arange(1, HD, 2)])
    Wq = W_qkv[0:D_MODEL].reshape(NUM_HEADS, HD, D_MODEL)[:, perm, :]
    Wk = W_qkv[D_MODEL : 2 * D_MODEL].reshape(NUM_HEADS, HD, D_MODEL)
    Wk = Wk[:, perm, :]
    Wv = W_qkv[2 * D_MODEL : 3 * D_MODEL].reshape(NUM_HEADS, HD, D_MODEL)

    mmdt = ml_dtypes.bfloat16
    in_maps = []
    for c in range(N_CORES):
        b, g = divmod(c, 4)
        h0, h1, h2 = 3 * g, 3 * g + 1, 3 * g + 2
        # col order: q0 q1 | k0 k1 | q2 k2 | v0 v1 v2
        w_c = np.concatenate(
            [Wq[h0], Wq[h1], Wk[h0], Wk[h1], Wq[h2], Wk[h2],
             Wv[h0], Wv[h1], Wv[h2]], axis=0)  # [576, 768]
        pos = np.asarray(token_positions[b], dtype=np.int64)
        in_maps.append({
            "xT": np.ascontiguousarray(x[b].T).astype(mmdt),
            "wqkvT": np.ascontiguousarray(w_c.T).astype(mmdt),
            "woT": np.ascontiguousarray(
                W_o[:, HG * g * HD : (HG * g + HG) * HD].T).astype(mmdt),
            "cosg": np.ascontiguousarray(cos_t[pos]).astype(mmdt),
            "sing": np.ascontiguousarray(sin_t[pos]).astype(mmdt),
        })
    return in_maps


def combine(partials):
    out = np.zeros((B, S, D_MODEL), dtype=np.float32)
    for c in range(N_CORES):
        out[c // 4] += np.asarray(partials[c], dtype=np.float32).T
    return out


def kernel(x, token_positions, W_qkv, W_o):
    nc = build_nc()
    in_maps = host_inputs(x, token_positions, W_qkv, W_o)
    res = run_bass_kernel_spmd(nc, in_maps, list(range(N_CORES)))
    return combine([res.results[c]["outT_partial"] for c in range(N_CORES)])


# revision 34
# speedup vs baseline: 1.1125x; 1.1125x over previous
"""Multi-head causal self-attention with RoPE on 8 Trainium2 NeuronCores.

Sharding: (batch, head-group) data+tensor parallel. Core c handles batch
c//4 and heads [3*(c%4), 3*(c%4)+3). Each core runs fused
QKV-projection + RoPE + causal attention + output-projection and emits a
partial transposed [D, S] output (fp16); the host sums the 4 head-group
partials per batch in fp32.

Device-side structure:
  - Q/K transposes to [d, s] go through the DMA X-bar: ONE [128,512]
    transpose per seq chunk fanning into a single [128, 4*S] tile of
    head-pair blocks (on the sync queue — a queued DMA holds its
    sequencer while waiting on deps, so it must not sit in front of
    the scalar engine's exps).
  - W_qkv columns are ordered (q0 q1 | k0 k1 | q2 k2 | v0 v1 v2) so the
    transposed blocks land as stacked head pairs; score matmuls for a
    pair run on PE row-groups 0-1/2-3 (K=64 row tiling) back-to-back;
    head 2 gets a swapped duplicate [k2;q2] so its blocks alternate row
    groups by k-block parity.
  - Score matmuls are emitted as <=256-column pieces: HW-measured
    per-matmul cost collapses from ~300ns (full 512-col fp32 bank) to
    ~40-100ns for sub-bank widths with alternating row groups.
  - RoPE runs in bf16 from an SBUF copy (DVE 16-bit rate) with bf16
    cos/sin tables.
  - Scores are computed transposed (S^T[k, q]); softmax skips
    max-subtraction; denominator comes free from a ones-column in V
    (M=65 PV matmuls — padding to M=128 measured slower, keep 65).
  - Causality: diagonal blocks compute only the needed (shrunk) column
    range; the partial triangle is zeroed AFTER exp in SBUF bf16 by
    GPSIMD affine_select restricted to the 128 columns where the mask
    has any effect (beyond 128 cols past the block diagonal, qi >= k
    always holds; fully-masked 128-col spans of a2's j=1 blocks are
    never read by PV, which starts at its own offset) — no mask work
    on the PSUM path and ~4x less than masking the whole block.
  - Output projection runs transposed: outT[d_model, s] accumulates
    Wo-chunk-stationary matmuls over ctxT slabs (ctx is already stored
    transposed), 2 matmuls per (qt, dmodel-group) unit; partials leave
    as [D, S] fp16 and the host transposes while summing.
  - Phase-1 chunks and phase-3 output-projection units are woven into
    the attention emission as PE filler, with each attention unit's
    tail (PV flush + normalize) deferred past the next unit's first
    k-blocks so neither PE nor ACT drains at unit boundaries.
  - P1 chunks drain just-in-time (p1late): chunk c is emitted ~one
    unit before its first consuming k-block instead of two units
    early. This spreads PE filler into the big late query tiles where
    the ACT engine is otherwise the rail, and was worth ~35us on HW
    (the early-flush schedule bunches p1 into the small early units
    and starves the qt=2/3 stretches).
  - The two reciprocal->broadcast->multiply normalize chains of an a1
    tail are emitted interleaved so their cross-engine hops pipeline.

HW-measured cost notes (this container, loop-delta timing):
  score matmuls in alternating 64-row groups genuinely overlap (two
  256-col pieces ~ cost of one); exp on ACT is ~1.33 ns/col (the
  ~82us exp rail and ~100us PE rail bound the kernel at ~105us ideal
  overlap); a cross-engine dependency round-trip is ~300-550ns, which
  is why emission order, not engine busy, dominates the schedule.
"""

import numpy as np

import concourse.bass as bass
import concourse.tile as tile
from concourse import bacc, mybir
from concourse._compat import with_exitstack
from concourse.bass_utils import run_bass_kernel_spmd

# Problem constants (hardcoded; kernel.py must be self-contained).
B = 2
S = 2048
D_MODEL = 768
NUM_HEADS = 12
HD = 64  # head dim
ROPE_THETA = 10000.0
MAX_SEQ_LEN = 2048

N_CORES = 8
HG = 3  # heads per core
E = 3 * HG * HD  # 576 qkv rows per core
P = 128
NSC = S // P  # 16 seq chunks
NKC = D_MODEL // P  # 6 d_model chunks
NG = D_MODEL // P  # 6 output d_model groups
F = HD // 2  # 32 rope freqs
QB = 512  # query block
NQT = S // QB  # 4 query tiles
VW = HD + 1  # V block width incl. ones column

F32 = mybir.dt.float32
F16 = mybir.dt.float16
MM = mybir.dt.bfloat16
EXP = mybir.ActivationFunctionType.Exp
GE = mybir.AluOpType.is_ge


@with_exitstack
def emit_mhsa(ctx, tc, loop_m=1, py_unroll=False, qkb_eng="act", ob_eng="dve",
              swap_eng="dve", parts="p1,att,p3", keep1=4, keep2=2, pace=2,
              pspool="shared", expsplit=0, odma="sync", obufs=6, rbufs=4,
              p1late=1, wide=0):
    parts = set(parts.split(","))
    nc = tc.nc
    xT = nc.dram_tensor("xT", [D_MODEL, S], MM, kind="ExternalInput").ap()
    wqkvT = nc.dram_tensor("wqkvT", [D_MODEL, E], MM, kind="ExternalInput").ap()
    woT = nc.dram_tensor("woT", [HG * HD, D_MODEL], MM, kind="ExternalInput").ap()
    cosg = nc.dram_tensor("cosg", [S, F], MM, kind="ExternalInput").ap()
    sing = nc.dram_tensor("sing", [S, F], MM, kind="ExternalInput").ap()
    out = nc.dram_tensor("outT_partial", [D_MODEL, S], F16, kind="ExternalOutput").ap()

    const = ctx.enter_context(tc.tile_pool(name="const", bufs=1))
    persist = ctx.enter_context(tc.tile_pool(name="persist", bufs=1))

    # ---- constants & weights ----
    cos_sb = const.tile([P, NSC * F], MM, tag="cos")
    sin_sb = const.tile([P, NSC * F], MM, tag="sin")
    nc.sync.dma_start(
        cos_sb[:].rearrange("p (n f) -> p n f", f=F),
        cosg.rearrange("(n p) f -> p n f", p=P),
    )
    nc.sync.dma_start(
        sin_sb[:].rearrange("p (n f) -> p n f", f=F),
        sing.rearrange("(n p) f -> p n f", p=P),
    )

    w_sb = []
    for kc in range(NKC):
        w = const.tile([P, E], MM, tag=f"wqkv{kc}", name=f"wqkv{kc}")
        nc.sync.dma_start(w[:], wqkvT[kc * P : (kc + 1) * P, :])
        w_sb.append(w)
    wo0 = const.tile([P, D_MODEL], MM, tag="wo0")
    wo1 = const.tile([HD, D_MODEL], MM, tag="wo1")
    nc.sync.dma_start(wo0[:], woT[0:P, :])
    nc.sync.dma_start(wo1[:], woT[P : HG * HD, :])

    x_sb = []
    for kc in range(NKC):
        xt = const.tile([P, S], MM, tag=f"x{kc}", name=f"x{kc}")
        nc.sync.dma_start(xt[:, 0:512], xT[kc * P : (kc + 1) * P, 0:512])
        x_sb.append(xt)
    for kc in range(NKC):
        nc.sync.dma_start(x_sb[kc][:, 512:S], xT[kc * P : (kc + 1) * P, 512:S])

    # ---- persistent intermediates ----
    # roped q/k in [d, s]: 4 stacked head-pair blocks in one tile:
    # t=0: [q0;q1]  t=1: [k0;k1]  t=2: [q2;k2]  t=3: [k2;q2]
    tqk = persist.tile([P, 4 * S], MM, tag="tqk")
    tqk4 = tqk[:].rearrange("p (t s) -> p t s", t=4)
    tq01 = tqk[:, 0:S]
    tk01 = tqk[:, S : 2 * S]
    tqk2 = tqk[:, 2 * S : 3 * S]
    tkq2 = tqk[:, 3 * S : 4 * S]
    v_sb = persist.tile([P, HG * NSC * VW], MM, tag="v")
    ctxA = persist.tile([P, S], MM, tag="ctxA")  # h0 rows 0:64, h1 rows 64:128
    ctxB = persist.tile([HD, S], MM, tag="ctxB")  # h2

    # ones columns for the PV denominator (V parts written each pass)
    v4 = v_sb[:].rearrange("p (h n w) -> p h n w", h=HG, n=NSC)
    nc.gpsimd.memset(v4[:, :, :, HD:VW], 1.0)
    if parts != {"p1", "att", "p3"}:  # timing probes read uninitialized tiles
        nc.gpsimd.memset(tqk[:], 0.01)
        nc.gpsimd.memset(v_sb[:], 0.01)
        nc.gpsimd.memset(ctxA[:], 0.01)
        nc.gpsimd.memset(ctxB[:], 0.01)

    if loop_m > 1 and not py_unroll:  # timing builds only: repeat compute body
        ctx.enter_context(tc.For_i(0, loop_m, 1))

    if pspool == "shared":
        ps_main = ctx.enter_context(
            tc.tile_pool(name="ps_main", bufs=3, space="PSUM"))
        ps_aux = ps_main
    else:  # scores get a dedicated pool; p1/p3 fillers can't starve them
        ps_main = ctx.enter_context(
            tc.tile_pool(name="ps_main", bufs=2, space="PSUM"))
        ps_aux = ctx.enter_context(
            tc.tile_pool(name="ps_aux", bufs=1, space="PSUM"))
    aux_tag = "ps" if pspool == "shared" else "psx"
    ps_ctx = ctx.enter_context(tc.tile_pool(name="ps_ctx", bufs=2, space="PSUM"))
    rope_pool = ctx.enter_context(tc.tile_pool(name="rope", bufs=rbufs))
    pp_pool = ctx.enter_context(tc.tile_pool(name="pp", bufs=8))
    norm_pool = ctx.enter_context(tc.tile_pool(name="norm", bufs=2))
    ob_pool = ctx.enter_context(tc.tile_pool(name="ob", bufs=obufs))
    odma_start = (nc.gpsimd.dma_start if odma == "pool"
                  else nc.sync.dma_start)

    qkb_copy = nc.scalar.copy if qkb_eng == "act" else nc.vector.tensor_copy
    ob_copy = nc.scalar.copy if ob_eng == "act" else nc.vector.tensor_copy
    swap_copy = (nc.gpsimd.tensor_copy if swap_eng == "pool"
                 else nc.vector.tensor_copy)

    # ================= emission building blocks =================

    def p1_chunk(sc):
        """QKV projection + RoPE + V copy + DMA-transpose for seq chunk sc."""
        pq = ps_aux.tile([P, 1024], F32, tag=aux_tag, name=f"p1_{sc}")
        pqk = pq[:, 0:384]  # bank 0
        pv = pq[:, 512:704]  # bank 1
        for kc in range(NKC):
            lhs = x_sb[kc][:, sc * P : (sc + 1) * P]
            st, sp = kc == 0, kc == NKC - 1
            nc.tensor.matmul(pqk, lhs, w_sb[kc][:, 0:384], start=st, stop=sp)
            nc.tensor.matmul(pv, lhs, w_sb[kc][:, 384:576], start=st, stop=sp)

        # V: strided copy into the 3 per-head blocks (+ones untouched).
        v_dst = v_sb[:].rearrange("p (h n w) -> p h n w", h=HG, n=NSC)
        nc.vector.tensor_copy(
            v_dst[:, :, sc, 0:HD], pv.rearrange("p (h w) -> p h w", h=HG)
        )

        # RoPE in bf16 (DVE 16-bit rate): one PSUM->SBUF copy, 6 DVE ops.
        qkb = rope_pool.tile([P, 384], MM, tag="qkb", name=f"qkb{sc}")
        qkb_copy(qkb[:], pqk)
        cos6 = (cos_sb[:, sc * F : (sc + 1) * F]
                .unsqueeze(1).broadcast_to([P, 6, F]))
        sin6 = (sin_sb[:, sc * F : (sc + 1) * F]
                .unsqueeze(1).broadcast_to([P, 6, F]))
        ro = rope_pool.tile([P, 512], MM, tag="ro", name=f"ro{sc}")
        r4 = ro[:, 0:384].rearrange("p (t two f) -> p t two f", t=6, two=2)
        s4 = qkb[:].rearrange("p (t two f) -> p t two f", t=6, two=2)
        ev, od = s4[:, :, 0, :], s4[:, :, 1, :]
        shape = [P, 6 * F]
        t1 = rope_pool.tile(shape, MM, tag="t1")
        t2 = rope_pool.tile(shape, MM, tag="t2")
        t14 = t1[:].rearrange("p (t f) -> p t f", t=6)
        t24 = t2[:].rearrange("p (t f) -> p t f", t=6)
        nc.vector.tensor_mul(t14, ev, cos6)
        nc.vector.tensor_mul(t24, od, sin6)
        nc.vector.tensor_sub(r4[:, :, 0, :], t14, t24)
        t3 = rope_pool.tile(shape, MM, tag="t3")
        t4 = rope_pool.tile(shape, MM, tag="t4")
        t34 = t3[:].rearrange("p (t f) -> p t f", t=6)
        t44 = t4[:].rearrange("p (t f) -> p t f", t=6)
        nc.vector.tensor_mul(t34, ev, sin6)
        nc.vector.tensor_mul(t44, od, cos6)
        nc.vector.tensor_add(r4[:, :, 1, :], t34, t44)

        # swapped duplicate [k2|q2] in cols 384:512
        swap_copy(ro[:, 384:448], ro[:, 320:384])
        swap_copy(ro[:, 448:512], ro[:, 256:320])

        # one [128,512] X-bar transpose into the 4 stacked blocks.
        # Stays on the sync queue: a queued DMA holds its sequencer while
        # waiting on deps, and the scalar queue must stay free for exps.
        nc.sync.dma_start(
            tqk4[:, :, sc * P : (sc + 1) * P], ro[:], transpose=True
        )

    def norm_heads(pcs_dsts):
        """dst = pctx[0:HD] / ones-row, pipelined across heads: both
        reciprocals issue first so the DVE->Pool->DVE chains overlap."""
        rs = []
        for pc, _ in pcs_dsts:
            rinv = norm_pool.tile([1, QB], F32, tag="rinv")
            nc.vector.reciprocal(rinv[0:1, :], pc[HD : HD + 1, :])
            rs.append(rinv)
        bs = []
        for rinv, (pc, _) in zip(rs, pcs_dsts):
            rbc = norm_pool.tile([HD, QB], F32, tag="rbc")
            nc.gpsimd.partition_broadcast(rbc[:], rinv[0:1, :])
            bs.append(rbc)
        for rbc, (pc, dst) in zip(bs, pcs_dsts):
            nc.vector.tensor_mul(dst, pc[0:HD, :], rbc[:])

    def a1_qt(qt, filler):
        """Heads 0,1: row-group-paired scores + exp + PV, one query tile.

        Generator: yields "kb" after each k-block and "pretail" before the
        PV flush + normalize, so the driver can start the next unit's
        scores/exps before this unit's tail work is queued.
        """
        nb = 4 * qt + 4
        pc0 = ps_ctx.tile([VW, QB], F32, tag="pctx", name=f"pc0_{qt}")
        pc1 = ps_ctx.tile([VW, QB], F32, tag="pctx", name=f"pc1_{qt}")
        pend = []

        def pv_flush(keep):
            while len(pend) > keep:
                psb, kb, off = pend.pop(0)
                for j, pc in ((0, pc0), (1, pc1)):
                    vb = (j * NSC + kb) * VW
                    nc.tensor.matmul(
                        pc[:, off:QB], v_sb[:, vb : vb + VW],
                        psb[:, j * QB + off : (j + 1) * QB],
                        start=(kb == 0), stop=(kb == nb - 1),
                    )

        for kb in range(nb):
            m = kb - 4 * qt
            off = 128 * m if m > 0 else 0
            w = QB - off
            ks = slice(kb * P, (kb + 1) * P)
            pss = ps_main.tile([P, 1024], F32, tag="ps", name=f"a1_{qt}_{kb}")
            # split into <=256-col pieces, alternating row groups per MM:
            # sub-bank-width matmuls issue far faster than full 512s.
            wa = w if wide else ((w + 255) // 256 * 128 if w > 256 else w)
            for o0, o1 in ((off, off + wa), (off + wa, QB)):
                if o0 >= o1:
                    continue
                qs = slice(qt * QB + o0, qt * QB + o1)
                nc.tensor.matmul(
                    pss[:, o0:o1], tk01[0:HD, ks], tq01[0:HD, qs],
                    start=True, stop=True,
                )
                nc.tensor.matmul(
                    pss[:, QB + o0 : QB + o1], tk01[HD:P, ks], tq01[HD:P, qs],
                    start=True, stop=True,
                )
            psb = pp_pool.tile([P, 1024], MM, tag="psb", name=f"e1_{qt}_{kb}")
            if m >= 0:
                src = pss[:].rearrange("p (h q) -> p h q", h=2)[:, :, off:QB]
                dst = psb[:].rearrange("p (h q) -> p h q", h=2)[:, :, off:QB]
                if expsplit:
                    nc.scalar.activation(dst[:, 0], src[:, 0], EXP, scale=0.125)
                    nc.scalar.activation(dst[:, 1], src[:, 1], EXP, scale=0.125)
                else:
                    nc.scalar.activation(dst, src, EXP, scale=0.125)
                # zero upper triangle in-place on GPSIMD (keep iff qi >= k).
                # Only the first 128 region cols can violate qi >= k.
                nc.gpsimd.affine_select(
                    out=dst[:, :, 0:P], in_=dst[:, :, 0:P], compare_op=GE,
                    fill=0.0, base=0, channel_multiplier=-1,
                    pattern=[[0, 2], [1, P]],
                )
            elif expsplit:
                nc.scalar.activation(psb[:, 0:QB], pss[:, 0:QB], EXP, scale=0.125)
                nc.scalar.activation(psb[:, QB:1024], pss[:, QB:1024], EXP,
                                     scale=0.125)
            else:
                nc.scalar.activation(psb[:], pss[:], EXP, scale=0.125)
            pend.append((psb, kb, off))
            pv_flush(keep1)
            filler(1)
            yield "kb"
        yield "pretail"
        pv_flush(0)
        norm_heads([(pc0, ctxA[0:HD, qt * QB : (qt + 1) * QB]),
                    (pc1, ctxA[HD:P, qt * QB : (qt + 1) * QB])])

    def a2_qt(qt, filler):
        """Head 2: k-block pairs alternate row groups by parity. Generator
        with the same yield protocol as a1_qt."""
        nb = 4 * qt + 4
        pc2 = ps_ctx.tile([VW, QB], F32, tag="pctx", name=f"pc2_{qt}")
        pend = []

        def pv_flush(keep):
            while len(pend) > keep:
                psb, kb0 = pend.pop(0)
                for j in (0, 1):
                    kb = kb0 + j
                    mj = kb - 4 * qt
                    offj = 128 * mj if mj > 0 else 0
                    vb = (2 * NSC + kb) * VW
                    nc.tensor.matmul(
                        pc2[:, offj:QB], v_sb[:, vb : vb + VW],
                        psb[:, j * QB + offj : (j + 1) * QB],
                        start=(kb == 0), stop=(kb == nb - 1),
                    )

        for kb0 in range(0, nb, 2):
            m0 = kb0 - 4 * qt
            off = 128 * m0 if m0 > 0 else 0
            w = QB - off
            pss = ps_main.tile([P, 1024], F32, tag="ps", name=f"a2_{qt}_{kb0}")
            wa = w if wide else ((w + 255) // 256 * 128 if w > 256 else w)
            for o0, o1 in ((off, off + wa), (off + wa, QB)):
                if o0 >= o1:
                    continue
                for j in (0, 1):
                    kb = kb0 + j
                    ks = slice(kb * P, (kb + 1) * P)
                    qs = slice(qt * QB + o0, qt * QB + o1)
                    if kb % 2 == 0:
                        lhsT, rhs = tkq2[0:HD, ks], tqk2[0:HD, qs]
                    else:
                        lhsT, rhs = tqk2[HD:P, ks], tkq2[HD:P, qs]
                    nc.tensor.matmul(
                        pss[:, j * QB + o0 : j * QB + o1], lhsT, rhs,
                        start=True, stop=True,
                    )
            psb = pp_pool.tile([P, 1024], MM, tag="psb", name=f"e2_{qt}_{kb0}")
            if m0 >= 0:
                src = pss[:].rearrange("p (h q) -> p h q", h=2)[:, :, off:QB]
                dst = psb[:].rearrange("p (h q) -> p h q", h=2)[:, :, off:QB]
                nc.scalar.activation(dst, src, EXP, scale=0.125)
                # j=0 block: triangle in region cols [0:128) (keep qi >= k).
                nc.gpsimd.affine_select(
                    out=dst[:, 0, 0:P], in_=dst[:, 0, 0:P], compare_op=GE,
                    fill=0.0, base=0, channel_multiplier=-1,
                    pattern=[[1, P]],
                )
                # j=1 block: its PV starts 128 cols later, so only region
                # cols [128:256) need the mask (keep qi-128 >= k there).
                nc.gpsimd.affine_select(
                    out=dst[:, 1, P : 2 * P], in_=dst[:, 1, P : 2 * P],
                    compare_op=GE, fill=0.0, base=0, channel_multiplier=-1,
                    pattern=[[1, P]],
                )
            else:
                nc.scalar.activation(psb[:], pss[:], EXP, scale=0.125)
            pend.append((psb, kb0))
            pv_flush(keep2)
            filler(1)
            yield "kb"
        yield "pretail"
        pv_flush(0)
        norm_heads([(pc2, ctxB[:, qt * QB : (qt + 1) * QB])])

    def p3_unit(qt, g):
        """Transposed output projection: outT[g-block, qt-slab]."""
        po = ps_aux.tile([P, 1024], F32, tag=aux_tag, name=f"p3_{qt}_{g}")
        gs = slice(g * P, (g + 1) * P)
        qs = slice(qt * QB, (qt + 1) * QB)
        nc.tensor.matmul(po[:, 0:QB], wo0[:, gs], ctxA[:, qs],
                         start=True, stop=False)
        nc.tensor.matmul(po[:, 0:QB], wo1[:, gs], ctxB[:, qs],
                         start=False, stop=True)
        ob = ob_pool.tile([P, QB], F16, tag="ob")
        ob_copy(ob[:], po[:, 0:QB])
        odma_start(out[gs, qs], ob[:])

    # ================= schedule =================
    # Filler queue: PE-heavy work woven between attention k-blocks so the
    # PE keeps running while ACT chews through the exps. P1 chunks have
    # deadlines (chunk c before attention unit c//4); P3 units become
    # available after their unit completes.
    def body():
        fillers = []

        def filler(budget):
            for _ in range(budget):
                if not fillers:
                    return
                fillers.pop(0)()

        def flush_p1_until(chunk_limit):
            while (fillers and fillers[0].__name__ == "p1"
                   and fillers[0].c < chunk_limit):
                fillers.pop(0)()

        def mk_p1(c):
            def p1():
                p1_chunk(c)
            p1.__name__ = "p1"
            p1.c = c
            return p1

        def mk_p3(qt, g):
            def p3():
                p3_unit(qt, g)
            p3.__name__ = "p3"
            return p3

        if "p1" in parts:
            for sc in range(4):
                p1_chunk(sc)
            fillers.extend(mk_p1(c) for c in range(4, NSC))

        every = [0]

        def paced_filler(_):
            every[0] += 1
            if every[0] % pace == 0:
                filler(1)

        if "att" not in parts:
            while fillers:
                fillers.pop(0)()
            if "p3" in parts:
                for qt_ in range(NQT):
                    for g_ in range(NG):
                        p3_unit(qt_, g_)
            return

        # Drive the attention units, overlapping each unit's tail (PV flush
        # + normalize) past the next unit's first k-blocks so ACT never
        # drains at a unit boundary. P3 units for a query tile only enter
        # the filler queue once the a2 tail (writing their ctxB) is emitted.
        pending_tail = None  # (generator, p3_qt or None)

        def finish(tail):
            g, p3_qt = tail
            for _ in g:
                pass
            if p3_qt is not None and "p3" in parts:
                fillers.extend(mk_p3(p3_qt, g_) for g_ in range(NG))

        for qt in range(NQT):
            # emit next unit's P1 chunks now: their QKV->RoPE->transpose
            # chain completes while this unit's attention runs.  With
            # p1late, chunks instead drain just-in-time (3 k-blocks ahead
            # of first use, enforced inside the kb loop) so PE filler work
            # spreads into the big late units instead of bunching early.
            flush_p1_until(4 * (qt + 1) if p1late else 4 * (qt + 2))
            for unit, p3_qt in (
                (a1_qt(qt, paced_filler), None),
                (a2_qt(qt, paced_filler), qt),
            ):
                steps = 0
                for ev in unit:
                    if ev == "kb":
                        steps += 1
                        if p1late and p3_qt is None:
                            flush_p1_until(4 * (qt + 1) + steps)
                        if steps == 2 and pending_tail is not None:
                            finish(pending_tail)
                            pending_tail = None
                    else:  # "pretail"
                        if pending_tail is not None:
                            finish(pending_tail)
                        pending_tail = (unit, p3_qt)
                        break
        if pending_tail is not None:
            finish(pending_tail)
        while fillers:
            fillers.pop(0)()

    if py_unroll:
        for _ in range(loop_m):
            body()
    else:
        body()


_NC_CACHE = None


def build_nc(loop_m=1, **kw):
    global _NC_CACHE
    key = (loop_m, tuple(sorted(kw.items())))
    if _NC_CACHE is None or getattr(_NC_CACHE, "_key", None) != key:
        nc = bacc.Bacc("TRN2", target_bir_lowering=False, debug=False)
        with tile.TileContext(nc) as tc:
            emit_mhsa(tc, loop_m=loop_m, **kw)
        nc.compile()
        nc._key = key
        _NC_CACHE = nc
    return _NC_CACHE


def _rope_tables():
    powers = np.arange(0, HD, 2, dtype=np.float32) / np.float32(HD)
    freqs = (1.0 / (ROPE_THETA ** powers)).astype(np.float32)
    t = np.arange(MAX_SEQ_LEN, dtype=np.float32)
    ang = t[:, None] * freqs[None, :]
    return np.cos(ang).astype(np.float32), np.sin(ang).astype(np.float32)


def host_inputs(x, token_positions, W_qkv, W_o):
    """Build the 8 per-core input maps (shard + layout prep)."""
    import ml_dtypes

    x = np.asarray(x, dtype=np.float32)
    token_positions = np.asarray(token_positions)
    W_qkv = np.asarray(W_qkv, dtype=np.float32)
    W_o = np.asarray(W_o, dtype=np.float32)

    cos_t, sin_t = _rope_tables()
    # De-interleave head-dim rows of W_q/W_k so RoPE pairs become
    # contiguous 32-wide halves on device.
    perm = np.concatenate([np.arange(0, HD, 2), np.arange(1, HD, 2)])
    Wq = W_qkv[0:D_MODEL].reshape(NUM_HEADS, HD, D_MODEL)[:, perm, :]
    Wk = W_qkv[D_MODEL : 2 * D_MODEL].reshape(NUM_HEADS, HD, D_MODEL)
    Wk = Wk[:, perm, :]
    Wv = W_qkv[2 * D_MODEL : 3 * D_MODEL].reshape(NUM_HEADS, HD, D_MODEL)

    mmdt = ml_dtypes.bfloat16
    in_maps = []
    for c in range(N_CORES):
        b, g = divmod(c, 4)
        h0, h1, h2 = 3 * g, 3 * g + 1, 3 * g + 2
        # col order: q0 q1 | k0 k1 | q2 k2 | v0 v1 v2
        w_c = np.concatenate(
            [Wq[h0], Wq[h1], Wk[h0], Wk[h1], Wq[h2], Wk[h2],
             Wv[h0], Wv[h1], Wv[h2]], axis=0)  # [576, 768]
        pos = np.asarray(token_positions[b], dtype=np.int64)
        in_maps.append({
            "xT": np.ascontiguousarray(x[b].T).astype(mmdt),
            "wqkvT": np.ascontiguousarray(w_c.T).astype(mmdt),
            "woT": np.ascontiguousarray(
                W_o[:, HG * g * HD : (HG * g + HG) * HD].T).astype(mmdt),
            "cosg": np.ascontiguousarray(cos_t[pos]).astype(mmdt),
            "sing": np.ascontiguousarray(sin_t[pos]).astype(mmdt),
        })
    return in_maps


def combine(partials):
    out = np.zeros((B, S, D_MODEL), dtype=np.float32)
    for c in range(N_CORES):
        out[c // 4] += np.asarray(partials[c], dtype=np.float32).T
    return out


def kernel(x, token_positions, W_qkv, W_o):
    nc = build_nc()
    in_maps = host_inputs(x, token_positions, W_qkv, W_o)
    res = run_bass_kernel_spmd(nc, in_maps, list(range(N_CORES)))
    return combine([res.results[c]["outT_partial"] for c in range(N_CORES)])
